# revision 7
# baseline (speedup 1.0000x reference)
"""Trainium2 Bass kernel for nn_CriticNetwork (gnn_message_passing).

Math: the reference GNN does mean-aggregation over a complete graph with
self-loops, so every node of an env sees the identical per-env mean.  The
whole network collapses to per-env scalars:

  m_b  = mean over the 16 nodes of obs[b]                      [128]
  p_b  = relu(m_b @ W1 + b1) @ W2 + b2                         [64]
  a_b  = p_b . (Wfc @ (Wattn[:64] + Wattn[64:]))               scalar
  w_b  = sigmoid(leaky_relu(a_b, 0.01))                        scalar
  c_b  = p_b . Wv[:64] + bv                                    scalar
  Q_bj = (act[b,j]-pi[b,j]) . Wvy ;  (Wvy = Wv[64:72])
  PS_b = sum_j pi[b,j].Wvy ;  QS_b = sum_j Q_bj
  xv[b,j] = c_b + PS_b/16 + w_b*(QS_b - Q_bj)/16
  out x[b*16+d, j] = xv[b,j]   (independent of d)
  out w[b*16+d, j] = w_b

Sharding: data-parallel over envs, 512 envs per core x 8 cores.

v3 design (chunked pipeline):
  - obs streams as 4 chunks of 512KB (128 envs each; partition p holds
    env 128k+p as 16 node rows = one 4KB contiguous line). Chunk 3 is
    split into two 256KB half-chunks on both rings so its last byte
    lands ~1.4us earlier. Measured: 2 HWDGE rings with >=512KB DMAs
    sustain ~350 GB/s aggregate; small chunks pay ~0.7us/trigger.
  - per chunk: node-sum tree (L1 on GpSimd for ch0-2 / DVE for ch3,
    L2-4 on DVE) -> meanS [128,128] -> PE transpose -> meanT (DVE copy)
    -> W1 matmul -> relu(+b1) on ACT -> wq matmul -> pwt[128,2] per
    chunk (a_raw, c_raw per env on partitions).
  - pol/act are host-folded to per-(env,node) scalars q = (act-pol).wvy
    and p = pol.wvy (extends the baseline's host *wvy fold), packed per
    the env->partition map: 32KB instead of 256KB, and the on-chip dot
    block shrinks to 3 tiny reduces/scales.
  - combine refactor: xv = w*A + B with A = QS/16 - q/16 (precomputed
    mid-stream) and B = PS/16 + b1v + c_raw -> the post-sigmoid tail is
    just 2 DVE ops + broadcast copies.
  - outputs per super-group (256 envs): wo/xo chunks issue as soon as
    ready; super-group 0 streams out mid-input; only xo1's completion
    receipt is exposed at the end.
  - consts are host-prepared bf16 (id128, W1/16, wq) + tiny f32 biases;
    no on-chip const casts.

Fixed framework overhead measured at ~9.5us (preamble + end barriers +
~7.3us semaphore-file zeroing epilogue); an empty kernel times ~13.8us.
"""

import numpy as np
import ml_dtypes
from contextlib import ExitStack

import concourse.bass as bass
import concourse.bacc as bacc
import concourse.tile as tile
from concourse import mybir
from concourse.bass_utils import run_bass_kernel_spmd

B, N, A = 4096, 16, 8
D_IN, H1, DP, DZ = 128, 64, 64, 64
NCORES = 8
BC = B // NCORES          # 512 envs per core
RC = BC * N               # 8192 obs rows per core
G = 4                     # chunks (128 envs each) per core
CBW = 194                 # bf16 const tile width: id128 | w1q | wq
CFW = 8                   # f32 const tile width: b1 | b0 | b1v | warm

F32 = mybir.dt.float32
BF16 = mybir.dt.bfloat16
ALU = mybir.AluOpType
AFT = mybir.ActivationFunctionType

USE_ACT_LRELU = True      # leaky-relu on ACT (alpha must be 0.01) vs DVE stt


def _build():
    nc = bacc.Bacc("TRN2", target_bir_lowering=False, debug=False)

    obs = nc.dram_tensor("obs", [RC, D_IN], BF16, kind="ExternalInput")
    qp = nc.dram_tensor("qp", [128, 128], BF16, kind="ExternalInput")
    cstb = nc.dram_tensor("cstb", [128, CBW], BF16, kind="ExternalInput")
    cstf = nc.dram_tensor("cstf", [128, CFW], F32, kind="ExternalInput")
    xo = nc.dram_tensor("xo", [RC, N], BF16, kind="ExternalOutput")
    wo = nc.dram_tensor("wo", [RC, N], BF16, kind="ExternalOutput")

    with ExitStack() as ctx:
        tc = ctx.enter_context(tile.TileContext(nc))
        consts = ctx.enter_context(tc.tile_pool(name="consts", bufs=1))
        obsp = ctx.enter_context(tc.tile_pool(name="obsp", bufs=1))
        trp = ctx.enter_context(tc.tile_pool(name="trp", bufs=2))
        chp = ctx.enter_context(tc.tile_pool(name="chp", bufs=2))
        smal = ctx.enter_context(tc.tile_pool(name="smal", bufs=2))
        outp = ctx.enter_context(tc.tile_pool(name="outp", bufs=1))
        pmtp = ctx.enter_context(tc.tile_pool(name="pmtp", bufs=2, space="PSUM"))
        php = ctx.enter_context(tc.tile_pool(name="php", bufs=2, space="PSUM"))
        pwtp = ctx.enter_context(tc.tile_pool(name="pwtp", bufs=2, space="PSUM"))

        # ---- input DMAs ------------------------------------------------
        # ring A (sync): ch0, ch2, ch3a + wo0, xo0, wo1 later
        # ring B (scalar): cstf, cstb+qp?, ch1, ch3b + xo1 later
        cf = consts.tile([128, CFW], F32)
        nc.scalar.dma_start(out=cf, in_=cstf.ap())
        cb = consts.tile([128, CBW], BF16)
        nc.scalar.dma_start(out=cb, in_=cstb.ap())
        qp_sb = consts.tile([128, 128], BF16)
        nc.scalar.dma_start(out=qp_sb, in_=qp.ap())

        # obs chunk views: partition p holds env 128k+p as a 4KB line
        ov = obs.ap().rearrange("(g p n) f -> g p (n f)", g=G, p=128, n=N)
        ch = [obsp.tile([128, N, D_IN], BF16, name=f"ch{k}") for k in range(G)]
        chv = lambda k: ch[k].rearrange("p n f -> p (n f)")
        nc.sync.dma_start(out=chv(0), in_=ov[0])
        nc.scalar.dma_start(out=chv(1), in_=ov[1])
        nc.sync.dma_start(out=chv(2), in_=ov[2])
        # chunk 3 split across both rings so its last byte lands earlier
        i_ch3a = nc.sync.dma_start(
            out=ch[3][:, 0:8, :].rearrange("p n f -> p (n f)"),
            in_=ov[3][:, 0:1024])
        i_ch3b = nc.scalar.dma_start(
            out=ch[3][:, 8:16, :].rearrange("p n f -> p (n f)"),
            in_=ov[3][:, 1024:2048])

        id128 = cb[:, 0:128]
        w1q = cb[:, 128:192]          # W1/16, bf16
        wq = cb[0:64, 192:194]        # W2 @ [wa | Wv64], bf16
        b1c = cf[:, 0:1]              # b1 on rows 0:64
        b0c = cf[:, 1:2]              # b2.wa broadcast
        b1v = cf[:, 2:3]              # b2.Wv64 + bv broadcast

        # warm the sigmoid table early (forces ACT_TABLE_LOAD up front)
        warm = consts.tile([1, 1], F32)
        nc.scalar.activation(out=warm, in_=cf[0:1, 3:4], func=AFT.Sigmoid)

        # ---- pol/act dot block (tiny, host-folded row sums) ------------
        q_v = qp_sb[:, 0:64].rearrange("p (g n) -> p g n", g=G)
        p_v = qp_sb[:, 64:128].rearrange("p (g n) -> p g n", g=G)
        QS4 = smal.tile([128, G], F32, name="QS4")
        nc.vector.reduce_sum(out=QS4, in_=q_v, axis=mybir.AxisListType.X)
        PS4 = smal.tile([128, G], F32, name="PS4")
        nc.vector.reduce_sum(out=PS4, in_=p_v, axis=mybir.AxisListType.X)
        # A[p,g,n] = QS4[p,g]/16 - q[p,g,n]/16
        QS4s = smal.tile([128, G], F32, name="QS4s")
        nc.vector.tensor_scalar_mul(QS4s, QS4, 1.0 / N)
        Abuf = smal.tile([128, G, N], F32, name="Abuf")
        nc.vector.scalar_tensor_tensor(
            out=Abuf, in0=q_v, scalar=-1.0 / N,
            in1=QS4s.unsqueeze(2).broadcast_to([128, G, N]),
            op0=ALU.mult, op1=ALU.add)
        # PS4s[p,g] = PS4/16 + b1v
        PS4s = smal.tile([128, G], F32, name="PS4s")
        nc.vector.scalar_tensor_tensor(
            out=PS4s, in0=PS4, scalar=1.0 / N,
            in1=b1v.broadcast_to([128, G]),
            op0=ALU.mult, op1=ALU.add)

        # ---- per-chunk chain -------------------------------------------
        def chunk_head(k):
            """node-sum tree -> meanS -> PE transpose -> meanT -> W1 mm ->
            relu -> wq mm.  Returns pwt PSUM slice holder via closure."""
            t = ch[k]
            if k < 3:
                s1 = trp.tile([128, 8, D_IN], BF16, name="s1")
                nc.gpsimd.tensor_add(s1, t[:, 0:8, :], t[:, 8:16, :])
            else:
                s1 = trp.tile([128, 8, D_IN], BF16, name="s1")
                nc.vector.tensor_add(s1, t[:, 0:8, :], t[:, 8:16, :])
            s2 = trp.tile([128, 4, D_IN], BF16, name="s2")
            nc.vector.tensor_add(s2, s1[:, 0:4, :], s1[:, 4:8, :])
            s3 = trp.tile([128, 2, D_IN], BF16, name="s3")
            nc.vector.tensor_add(s3, s2[:, 0:2, :], s2[:, 2:4, :])
            meanS = trp.tile([128, D_IN], BF16, name="meanS")
            nc.vector.tensor_add(meanS, s3[:, 0, :], s3[:, 1, :])
            pmt = pmtp.tile([128, 128], BF16, name="pmt")
            nc.tensor.transpose(pmt, meanS, id128)
            meanT = chp.tile([128, 128], BF16, name="meanT")
            if k == 3:
                nc.scalar.activation(out=meanT, in_=pmt, func=AFT.Copy)
            else:
                nc.vector.tensor_copy(meanT, pmt)
            ph = php.tile([64, 128], F32, name="ph")
            nc.tensor.matmul(ph, lhsT=w1q, rhs=meanT, start=True, stop=True)
            h_sb = chp.tile([64, 128], BF16, name="h_sb")
            nc.scalar.activation(out=h_sb, in_=ph, func=AFT.Relu, bias=b1c[0:64])
            return h_sb

        def supergroup(j, pwt):
            """post-matmul scalar chain + combine + output copies for the
            256 envs of chunks {2j, 2j+1}; pwt is [128, 2, 2] PSUM."""
            # w-column: leaky_relu(a + b0) then sigmoid
            wl = smal.tile([128, 2, 1], F32, name="wl")
            if USE_ACT_LRELU:
                nc.scalar.activation(out=wl, in_=pwt[:, :, 0:1], func=AFT.Lrelu,
                                     bias=b0c)
            else:
                wb = smal.tile([128, 2, 1], F32, name="wb")
                nc.vector.tensor_add(wb, pwt[:, :, 0:1],
                                     b0c.unsqueeze(1).broadcast_to([128, 2, 1]))
                nc.vector.scalar_tensor_tensor(out=wl, in0=wb, scalar=0.01,
                                               in1=wb, op0=ALU.mult, op1=ALU.max)
            sig_i = nc.scalar.activation(out=wl, in_=wl, func=AFT.Sigmoid)
            # B-column: PS4s + c_raw (DVE reads PSUM)
            Bt = smal.tile([128, 2, 1], F32, name="Bt")
            nc.vector.tensor_add(Bt, pwt[:, :, 1:2],
                                 PS4s[:, 2 * j:2 * j + 2].unsqueeze(2))
            # xv = wl*A + B
            m = smal.tile([128, 2, N], F32, name="m")
            nc.vector.tensor_mul(m, Abuf[:, 2 * j:2 * j + 2, :],
                                 wl.broadcast_to([128, 2, N]))
            xv = smal.tile([128, 2, N], F32, name="xv")
            nc.vector.tensor_add(xv, m, Bt.broadcast_to([128, 2, N]))
            # broadcast payloads
            wbig = outp.tile([128, 2 * N * N], BF16, name=f"wbig{j}")
            nc.scalar.activation(
                out=wbig.rearrange("p (g dj) -> p g dj", g=2),
                in_=wl.broadcast_to([128, 2, N * N]), func=AFT.Copy)
            xbig = outp.tile([128, 2 * N * N], BF16, name=f"xbig{j}")
            nc.vector.tensor_copy(
                xbig.rearrange("p (g d j) -> p g d j", g=2, d=N),
                xv.unsqueeze(2).broadcast_to([128, 2, N, N]))
            return sig_i, wbig, xbig

        wo_v = wo.ap().rearrange("(jj g2 p d) j -> jj p g2 (d j)",
                                 jj=2, g2=2, p=128, d=N)
        xo_v = xo.ap().rearrange("(jj g2 p d) j -> jj p g2 (d j)",
                                 jj=2, g2=2, p=128, d=N)

        out_trigs = []
        for j in range(2):
            pwt = pwtp.tile([128, 2, 2], F32, name=f"pwt{j}")
            for g2 in range(2):
                h_sb = chunk_head(2 * j + g2)
                nc.tensor.matmul(pwt[:, g2, :], lhsT=h_sb, rhs=wq,
                                 start=True, stop=True)
            sig_i, wbig, xbig = supergroup(j, pwt)
            wb4 = wbig.rearrange("p (g2 dj) -> p g2 dj", g2=2)
            xb4 = xbig.rearrange("p (g2 dj) -> p g2 dj", g2=2)
            if j == 0:
                out_trigs.append(nc.sync.dma_start(out=wo_v[0], in_=wb4))
                out_trigs.append(nc.sync.dma_start(out=xo_v[0], in_=xb4))
            else:
                out_trigs.append(nc.sync.dma_start(out=wo_v[1], in_=wb4))
                i_xo1 = nc.scalar.dma_start(out=xo_v[1], in_=xb4)

        # keep output triggers behind the last input trigger on each ring
        # so the scheduler cannot stall the input stream on them
        prev = i_ch3a
        for di in out_trigs:
            tile.add_dep_helper(di.ins, prev.ins, sync=False,
                                reason="sync outputs after inputs, in order")
            prev = di
        tile.add_dep_helper(i_xo1.ins, i_ch3b.ins, sync=False,
                            reason="scalar output after inputs")

    nc.compile()
    return nc


_NC_CACHE = {}


def _get_nc():
    if "nc" not in _NC_CACHE:
        _NC_CACHE["nc"] = _build()
    return _NC_CACHE["nc"]


def _make_in_maps(inputs):
    bf = ml_dtypes.bfloat16
    obs = np.ascontiguousarray(np.asarray(inputs["obs"], np.float32)).astype(bf)
    pol0 = np.asarray(inputs["policies"], np.float32)
    act0 = np.asarray(inputs["actions"], np.float32)
    W1 = np.asarray(inputs["W1"], np.float32)
    b1 = np.asarray(inputs["b1"], np.float32)
    W2 = np.asarray(inputs["W2"], np.float32)
    b2 = np.asarray(inputs["b2"], np.float32)
    Wfc = np.asarray(inputs["Wfc"], np.float32)
    Wattn = np.asarray(inputs["Wattn"], np.float32)
    Wv = np.asarray(inputs["Wv"], np.float32)
    bv = np.asarray(inputs["bv"], np.float32)

    wa = (Wfc @ (Wattn[:DZ] + Wattn[DZ:]))[:, 0]     # [64]
    wvy = Wv[DP:, 0]                                  # [8]
    wv64 = Wv[:DP, 0]

    # host-folded per-(env,node) dot scalars
    qv = ((act0 - pol0) * wvy).sum(-1)                # [B*N] f32
    pv = (pol0 * wvy).sum(-1)                         # [B*N]

    cstb = np.zeros((128, CBW), np.float32)
    cstb[:, 0:128] = np.eye(128, dtype=np.float32)
    cstb[:, 128:192] = W1 / float(N)
    cstb[0:64, 192] = W2 @ wa
    cstb[0:64, 193] = W2 @ wv64
    cstb = cstb.astype(bf)

    cstf = np.zeros((128, CFW), np.float32)
    cstf[0:64, 0] = b1
    cstf[:, 1] = float(b2 @ wa)
    cstf[:, 2] = float(b2 @ wv64 + bv[0])

    in_maps = []
    for c in range(NCORES):
        # qp[p, t, g, n]: q/p value of env 128g+p, node n (within core c)
        qc = qv[c * RC:(c + 1) * RC].reshape(G, 128, N).transpose(1, 0, 2)
        pc = pv[c * RC:(c + 1) * RC].reshape(G, 128, N).transpose(1, 0, 2)
        qp_c = np.concatenate(
            [qc.reshape(128, 64), pc.reshape(128, 64)], axis=1).astype(bf)
        in_maps.append({
            "obs": obs[c * RC:(c + 1) * RC],
            "qp": np.ascontiguousarray(qp_c),
            "cstb": cstb,
            "cstf": cstf,
        })
    return in_maps


# Test-harness knobs (the grader just calls kernel() with defaults).
TRACE = False
TRACE_KWARGS = {}
LAST_RESULT = None


def kernel(**inputs):
    global LAST_RESULT
    nc = _get_nc()
    in_maps = _make_in_maps(inputs)
    res = run_bass_kernel_spmd(nc, in_maps, core_ids=list(range(NCORES)),
                               trace=TRACE, **TRACE_KWARGS)
    LAST_RESULT = res
    x = np.concatenate([np.asarray(r["xo"], np.float32)
                        for r in res.results], axis=0).reshape(B * N, N, 1)
    w = np.concatenate([np.asarray(r["wo"], np.float32)
                        for r in res.results], axis=0).reshape(B * N, N, 1)
    return x, w


# revision 11
# speedup vs baseline: 1.1914x; 1.1914x over previous
"""Trainium2 Bass kernel for nn_CriticNetwork (gnn_message_passing).

Math: the reference GNN does mean-aggregation over a complete graph with
self-loops, so every node of an env sees the identical per-env mean.  The
whole network collapses to per-env scalars:

  m_b  = mean over the 16 nodes of obs[b]                      [128]
  p_b  = relu(m_b @ W1 + b1) @ W2 + b2                         [64]
  a_b  = p_b . (Wfc @ (Wattn[:64] + Wattn[64:]))               scalar
  w_b  = sigmoid(leaky_relu(a_b, 0.01))                        scalar
  c_b  = p_b . Wv[:64] + bv                                    scalar
  Q_bj = (act[b,j]-pi[b,j]) . Wvy ;  (Wvy = Wv[64:72])
  PS_b = sum_j pi[b,j].Wvy ;  QS_b = sum_j Q_bj
  xv[b,j] = c_b + PS_b/16 + w_b*(QS_b - Q_bj)/16
  out x[b*16+d, j] = xv[b,j]   (independent of d)
  out w[b*16+d, j] = w_b

Sharding: data-parallel over envs, 512 envs per core x 8 cores.

v3.1 design (chunked pipeline):
  - obs streams as 4 compute chunks of 128 envs (partition p holds env
    128k+p as 16 node rows = one 4KB contiguous HBM line).  Chunks 2,3
    are split into node-halves across both HWDGE rings so ring byte
    loads are equal (~1.05MB each) and the last chunk's halves land
    together.  Measured: >=256KB DMAs on 2 rings sustain ~350 GB/s
    aggregate; each trigger instruction costs ~0.7us of ring time.
  - per chunk: node-sum tree (L1/L3/L4 on DVE, L2 on GpSimd for chunks
    0-2; chunk 3 all-DVE for minimal tail latency) -> meanS[128,128]
    -> PE transpose -> meanT (DVE copy, ACT for ch3) -> W1/16 matmul
    -> relu(+b1) on ACT -> [h;1] @ wq_aug matmul -> pwt[128,2]
    (a+b0, c+b1v per env on partitions; biases ride the ones-row).
  - pol/act are host-folded to per-(env,node) scalars q=(act-pol).wvy/16
    and p=pol.wvy/16 (extends the baseline's host *wvy fold): 32KB
    streamed instead of 256KB, and the on-chip dot block is 3 DVE ops:
    QS=reduce(q), PS=reduce(p), A = QS - q.
  - combine: xv = w*A + B with B = PS + c_col (DVE reads PSUM), then
    broadcast copies to the 16x-redundant output layout.
  - outputs per super-group (256 envs) issue as soon as ready; only the
    last xo chunk's completion receipt is exposed at the end.
  - consts host-prepared bf16 (id128, W1/16, wq_aug) + tiny f32 col of
    ones; no on-chip const casts; no Lrelu ACT func (different table
    set -> reload storm); leaky-relu is one DVE scalar_tensor_tensor.

Fixed framework overhead ~9.5us (preamble + end barriers + ~7.3us
semaphore-file zeroing epilogue); an empty kernel measures ~13.8us.
"""

import numpy as np
import ml_dtypes
from contextlib import ExitStack

import concourse.bass as bass
import concourse.bacc as bacc
import concourse.tile as tile
from concourse import mybir
from concourse.bass_utils import run_bass_kernel_spmd

B, N, A = 4096, 16, 8
D_IN, H1, DP, DZ = 128, 64, 64, 64
NCORES = 8
BC = B // NCORES          # 512 envs per core
RC = BC * N               # 8192 obs rows per core
G = 4                     # chunks (128 envs each) per core
CBW = 194                 # bf16 const cols: id128 | w1q | wq_aug
CFW = 4                   # f32 const cols: b1 | ones | spare

F32 = mybir.dt.float32
BF16 = mybir.dt.bfloat16
ALU = mybir.AluOpType
AFT = mybir.ActivationFunctionType


def _build():
    nc = bacc.Bacc("TRN2", target_bir_lowering=False, debug=False)

    obs = nc.dram_tensor("obs", [RC, D_IN], BF16, kind="ExternalInput")
    qp = nc.dram_tensor("qp", [128, 128], BF16, kind="ExternalInput")
    cstb = nc.dram_tensor("cstb", [128, CBW], BF16, kind="ExternalInput")
    cstf = nc.dram_tensor("cstf", [128, CFW], F32, kind="ExternalInput")
    xo = nc.dram_tensor("xo", [RC, N], BF16, kind="ExternalOutput")
    wo = nc.dram_tensor("wo", [RC, N], BF16, kind="ExternalOutput")

    with ExitStack() as ctx:
        tc = ctx.enter_context(tile.TileContext(nc))
        consts = ctx.enter_context(tc.tile_pool(name="consts", bufs=1))
        obsp = ctx.enter_context(tc.tile_pool(name="obsp", bufs=1))
        trp = ctx.enter_context(tc.tile_pool(name="trp", bufs=2))
        chp = ctx.enter_context(tc.tile_pool(name="chp", bufs=2))
        smal = ctx.enter_context(tc.tile_pool(name="smal", bufs=2))
        outp = ctx.enter_context(tc.tile_pool(name="outp", bufs=1))
        pmtp = ctx.enter_context(tc.tile_pool(name="pmtp", bufs=2, space="PSUM"))
        php = ctx.enter_context(tc.tile_pool(name="php", bufs=2, space="PSUM"))
        pwtp = ctx.enter_context(tc.tile_pool(name="pwtp", bufs=2, space="PSUM"))

        # ---- input DMAs ------------------------------------------------
        # ring A (sync):   ch0, ch2a, ch3a            (~1.00 MB)
        # ring B (scalar): cstf, cstb, qp, ch1, ch2b, ch3b  (~1.11 MB)
        cf = consts.tile([128, CFW], F32)
        nc.scalar.dma_start(out=cf, in_=cstf.ap())
        cb = consts.tile([128, CBW], BF16)
        nc.scalar.dma_start(out=cb, in_=cstb.ap())
        qp_sb = consts.tile([128, 128], BF16)
        nc.scalar.dma_start(out=qp_sb, in_=qp.ap())

        # obs chunk views: partition p holds env 128k+p as a 4KB line
        ov = obs.ap().rearrange("(g p n) f -> g p (n f)", g=G, p=128, n=N)
        ch = [obsp.tile([128, N, D_IN], BF16, name=f"ch{k}") for k in range(G)]
        flat = lambda t: t.rearrange("p n f -> p (n f)")
        half = lambda k, h: ch[k][:, 8 * h:8 * h + 8, :].rearrange(
            "p n f -> p (n f)")
        nc.sync.dma_start(out=flat(ch[0]), in_=ov[0])
        nc.scalar.dma_start(out=flat(ch[1]), in_=ov[1])
        nc.sync.dma_start(out=half(2, 0), in_=ov[2][:, 0:1024])
        nc.scalar.dma_start(out=half(2, 1), in_=ov[2][:, 1024:2048])
        i_ch3a = nc.sync.dma_start(out=half(3, 0), in_=ov[3][:, 0:1024])
        i_ch3b = nc.scalar.dma_start(out=half(3, 1), in_=ov[3][:, 1024:2048])

        id128 = cb[:, 0:128]
        w1q = cb[:, 128:192]          # W1/16, bf16
        wq = cb[0:65, 192:194]        # [W2@wa | W2@wv64 ; b0 | b1v], bf16
        b1c = cf[:, 0:1]              # b1 on rows 0:64 (f32)
        onec = cf[:, 1:2]             # 1.0 everywhere (f32)

        # warm the sigmoid table early (forces ACT_TABLE_LOAD up front)
        warm = consts.tile([1, 1], F32)
        nc.scalar.activation(out=warm, in_=cf[0:1, 1:2], func=AFT.Sigmoid)

        # h tiles carry a ones-row (row 64) so wq_aug applies the biases;
        # initialize both ring buffers once, off the critical path.
        h_tiles = [chp.tile([65, 128], BF16, name="h_sb") for _ in range(2)]
        for t in h_tiles:
            nc.vector.tensor_copy(t[64:65, :],
                                  onec[64:65, :].broadcast_to([1, 128]))

        # ---- per-chunk chain -------------------------------------------
        def tree(k):
            t = ch[k]
            s1 = trp.tile([128, 8, D_IN], BF16, name="s1")
            nc.vector.tensor_add(s1, t[:, 0:8, :], t[:, 8:16, :])
            s2 = trp.tile([128, 4, D_IN], BF16, name="s2")
            if k < 3:
                nc.gpsimd.tensor_add(s2, s1[:, 0:4, :], s1[:, 4:8, :])
            else:
                nc.vector.tensor_add(s2, s1[:, 0:4, :], s1[:, 4:8, :])
            s3 = trp.tile([128, 2, D_IN], BF16, name="s3")
            nc.vector.tensor_add(s3, s2[:, 0:2, :], s2[:, 2:4, :])
            meanS = trp.tile([128, D_IN], BF16, name="meanS")
            nc.vector.tensor_add(meanS, s3[:, 0, :], s3[:, 1, :])
            pmt = pmtp.tile([128, 128], BF16, name="pmt")
            nc.tensor.transpose(pmt, meanS, id128)
            meanT = chp.tile([128, 128], BF16, name="meanT")
            if k == 3:
                nc.scalar.activation(out=meanT, in_=pmt, func=AFT.Copy)
            else:
                nc.vector.tensor_copy(meanT, pmt)
            return meanT

        def head(k, meanT, pwt, g2):
            ph = php.tile([64, 128], F32, name="ph")
            nc.tensor.matmul(ph, lhsT=w1q, rhs=meanT, start=True, stop=True)
            h_sb = h_tiles[k % 2]
            nc.scalar.activation(out=h_sb[0:64, :], in_=ph, func=AFT.Relu,
                                 bias=b1c[0:64])
            nc.tensor.matmul(pwt[:, g2, :], lhsT=h_sb, rhs=wq,
                             start=True, stop=True)

        # ---- emission, ordered for per-engine queues -------------------
        pwt0 = pwtp.tile([128, 2, 2], F32, name="pwt0")
        pwt1 = pwtp.tile([128, 2, 2], F32, name="pwt1")

        mT = tree(0)
        head(0, mT, pwt0, 0)
        mT = tree(1)
        head(1, mT, pwt0, 1)

        # dot block (after ch0/ch1 trees in the DVE queue so the early
        # stream isn't stalled on qp's completion)
        q_v = qp_sb[:, 0:64].rearrange("p (g n) -> p g n", g=G)
        p_v = qp_sb[:, 64:128].rearrange("p (g n) -> p g n", g=G)
        QS4 = smal.tile([128, G], F32, name="QS4")
        nc.vector.reduce_sum(out=QS4, in_=q_v, axis=mybir.AxisListType.X)
        PS4 = smal.tile([128, G], F32, name="PS4")
        nc.vector.reduce_sum(out=PS4, in_=p_v, axis=mybir.AxisListType.X)
        Abuf = smal.tile([128, G, N], F32, name="Abuf")
        nc.vector.scalar_tensor_tensor(
            out=Abuf, in0=q_v, scalar=-1.0,
            in1=QS4.unsqueeze(2).broadcast_to([128, G, N]),
            op0=ALU.mult, op1=ALU.add)

        wo_v = wo.ap().rearrange("(jj g2 p d) j -> jj p g2 (d j)",
                                 jj=2, g2=2, p=128, d=N)
        xo_v = xo.ap().rearrange("(jj g2 p d) j -> jj p g2 (d j)",
                                 jj=2, g2=2, p=128, d=N)

        def supergroup(j, pwt):
            # PSUM -> SBUF once, then leaky-relu on DVE (one stt), sigmoid
            # on ACT (an instruction may read PSUM through at most one
            # non-scalar input)
            pw4 = smal.tile([128, 2, 2], F32, name="pw4")
            nc.vector.tensor_copy(pw4, pwt)
            wl = smal.tile([128, 2, 1], F32, name="wl")
            nc.vector.scalar_tensor_tensor(out=wl, in0=pw4[:, :, 0:1],
                                           scalar=0.01, in1=pw4[:, :, 0:1],
                                           op0=ALU.mult, op1=ALU.max)
            nc.scalar.activation(out=wl, in_=wl, func=AFT.Sigmoid)
            Bt = smal.tile([128, 2, 1], F32, name="Bt")
            nc.vector.tensor_add(Bt, pw4[:, :, 1:2],
                                 PS4[:, 2 * j:2 * j + 2].unsqueeze(2))
            m = smal.tile([128, 2, N], F32, name="m")
            nc.vector.tensor_mul(m, Abuf[:, 2 * j:2 * j + 2, :],
                                 wl.broadcast_to([128, 2, N]))
            xv = smal.tile([128, 2, N], F32, name="xv")
            nc.vector.tensor_add(xv, m, Bt.broadcast_to([128, 2, N]))
            xbig = outp.tile([128, 2 * N * N], BF16, name=f"xbig{j}")
            nc.vector.tensor_copy(
                xbig.rearrange("p (g d j) -> p g d j", g=2, d=N),
                xv.unsqueeze(2).broadcast_to([128, 2, N, N]))
            wbig = outp.tile([128, 2 * N * N], BF16, name=f"wbig{j}")
            nc.vector.tensor_copy(wbig.rearrange("p (g dj) -> p g dj", g=2),
                                  wl.broadcast_to([128, 2, N * N]))
            return wbig, xbig

        wbig0, xbig0 = supergroup(0, pwt0)
        i_wo0 = nc.sync.dma_start(
            out=wo_v[0], in_=wbig0.rearrange("p (g2 dj) -> p g2 dj", g2=2))
        i_xo0 = nc.sync.dma_start(
            out=xo_v[0], in_=xbig0.rearrange("p (g2 dj) -> p g2 dj", g2=2))

        mT = tree(2)
        head(2, mT, pwt1, 0)
        mT = tree(3)
        head(3, mT, pwt1, 1)

        wbig1, xbig1 = supergroup(1, pwt1)
        i_wo1 = nc.sync.dma_start(
            out=wo_v[1], in_=wbig1.rearrange("p (g2 dj) -> p g2 dj", g2=2))
        i_xo1 = nc.scalar.dma_start(
            out=xo_v[1], in_=xbig1.rearrange("p (g2 dj) -> p g2 dj", g2=2))

        # keep output triggers behind the last input trigger on each ring
        prev = i_ch3a
        for di in (i_wo0, i_xo0, i_wo1):
            tile.add_dep_helper(di.ins, prev.ins, sync=False,
                                reason="sync outputs after inputs, in order")
            prev = di
        tile.add_dep_helper(i_xo1.ins, i_ch3b.ins, sync=False,
                            reason="scalar output after inputs")

    nc.compile()
    return nc


_NC_CACHE = {}


def _get_nc():
    if "nc" not in _NC_CACHE:
        _NC_CACHE["nc"] = _build()
    return _NC_CACHE["nc"]


def _make_in_maps(inputs):
    bf = ml_dtypes.bfloat16
    obs = np.ascontiguousarray(np.asarray(inputs["obs"], np.float32)).astype(bf)
    pol0 = np.asarray(inputs["policies"], np.float32)
    act0 = np.asarray(inputs["actions"], np.float32)
    W1 = np.asarray(inputs["W1"], np.float32)
    b1 = np.asarray(inputs["b1"], np.float32)
    W2 = np.asarray(inputs["W2"], np.float32)
    b2 = np.asarray(inputs["b2"], np.float32)
    Wfc = np.asarray(inputs["Wfc"], np.float32)
    Wattn = np.asarray(inputs["Wattn"], np.float32)
    Wv = np.asarray(inputs["Wv"], np.float32)
    bv = np.asarray(inputs["bv"], np.float32)

    wa = (Wfc @ (Wattn[:DZ] + Wattn[DZ:]))[:, 0]     # [64]
    wvy = Wv[DP:, 0]                                  # [8]
    wv64 = Wv[:DP, 0]

    # host-folded per-(env,node) dot scalars, pre-divided by N
    qv = ((act0 - pol0) * wvy).sum(-1) / float(N)     # [B*N] f32
    pv = (pol0 * wvy).sum(-1) / float(N)

    cstb = np.zeros((128, CBW), np.float32)
    cstb[:, 0:128] = np.eye(128, dtype=np.float32)
    cstb[:, 128:192] = W1 / float(N)
    cstb[0:64, 192] = W2 @ wa
    cstb[0:64, 193] = W2 @ wv64
    cstb[64, 192] = float(b2 @ wa)            # b0 via ones-row
    cstb[64, 193] = float(b2 @ wv64 + bv[0])  # b1v via ones-row
    cstb = cstb.astype(bf)

    cstf = np.zeros((128, CFW), np.float32)
    cstf[0:64, 0] = b1
    cstf[:, 1] = 1.0

    in_maps = []
    for c in range(NCORES):
        # qp[p, t, g, n]: q/p value of env 128g+p, node n (within core c)
        qc = qv[c * RC:(c + 1) * RC].reshape(G, 128, N).transpose(1, 0, 2)
        pc = pv[c * RC:(c + 1) * RC].reshape(G, 128, N).transpose(1, 0, 2)
        qp_c = np.concatenate(
            [qc.reshape(128, 64), pc.reshape(128, 64)], axis=1).astype(bf)
        in_maps.append({
            "obs": obs[c * RC:(c + 1) * RC],
            "qp": np.ascontiguousarray(qp_c),
            "cstb": cstb,
            "cstf": cstf,
        })
    return in_maps


# Test-harness knobs (the grader just calls kernel() with defaults).
TRACE = False
TRACE_KWARGS = {}
LAST_RESULT = None


def kernel(**inputs):
    global LAST_RESULT
    nc = _get_nc()
    in_maps = _make_in_maps(inputs)
    res = run_bass_kernel_spmd(nc, in_maps, core_ids=list(range(NCORES)),
                               trace=TRACE, **TRACE_KWARGS)
    LAST_RESULT = res
    x = np.concatenate([np.asarray(r["xo"], np.float32)
                        for r in res.results], axis=0).reshape(B * N, N, 1)
    w = np.concatenate([np.asarray(r["wo"], np.float32)
                        for r in res.results], axis=0).reshape(B * N, N, 1)
    return x, w


# revision 16
# speedup vs baseline: 1.3298x; 1.1162x over previous
"""Trainium2 Bass kernel for nn_CriticNetwork (gnn_message_passing).

Math: the reference GNN does mean-aggregation over a complete graph with
self-loops, so every node of an env sees the identical per-env mean.  The
whole network collapses to per-env scalars:

  m_b  = mean over the 16 nodes of obs[b]                      [128]
  p_b  = relu(m_b @ W1 + b1) @ W2 + b2                         [64]
  a_b  = p_b . (Wfc @ (Wattn[:64] + Wattn[64:]))               scalar
  w_b  = sigmoid(leaky_relu(a_b, 0.01))                        scalar
  c_b  = p_b . Wv[:64] + bv                                    scalar
  Q_bj = (act[b,j]-pi[b,j]) . Wvy ;  (Wvy = Wv[64:72])
  PS_b = sum_j pi[b,j].Wvy ;  QS_b = sum_j Q_bj
  xv[b,j] = c_b + PS_b/16 + w_b*(QS_b - Q_bj)/16
  out x[b*16+d, j] = xv[b,j]   (independent of d)
  out w[b*16+d, j] = w_b

Sharding: data-parallel over envs, 512 envs per core x 8 cores.

v3.2 design (chunked pipeline):
  - ONE merged const+qp DMA (id128 | W1/16 | wq_aug | b1 | ones | q | p,
    all bf16) so ring B has only 4 trigger instructions (~0.65us each on
    the issuing engine -- 7 triggers serialized to 12us in v3.1).
  - obs streams as 4 compute chunks of 128 envs (partition p holds env
    128k+p as 16 node rows = one 4KB HBM line); chunks 2,3 split into
    node-halves across both rings to balance ring bytes (~1.06MB each).
  - per chunk: DVE tree levels s1,s2 (s3 too for ch3), then the
    remaining node-sum happens as ACCUMULATING PE transposes into PSUM
    (is_transpose matmuls with start/stop), absorbing tree levels into
    the idle TensorE; ACT copies meanT out, W1/16 matmul, relu(+b1),
    [h;1] @ wq_aug matmul -> pwt[128,2] (biases ride the ones-row).
  - pol/act host-folded to q=(act-pol).wvy/16, p=pol.wvy/16 per
    (env,node) (extends the baseline's host *wvy fold): 32KB streamed
    instead of 256KB; dot block (QS/PS reduce + A = QS - q) on GpSimd.
  - combine: xv = w*A + B, B = PS + c_col; leaky-relu is tensor_scalar
    + max (PSUM-legal, no ACT Lrelu -> avoids table-reload storm);
    supergroup 0 smalls on GpSimd, supergroup 1 on DVE (tail).
  - outputs per super-group issue as soon as ready on ring A; only the
    last xo chunk's completion receipt is exposed.

Fixed framework overhead ~9.5us (preamble + end barriers + ~7.3us
semaphore-file zeroing epilogue); an empty kernel measures ~13.8us.
DMA facts measured on this part: 2 HWDGE rings sustain ~350 GB/s
aggregate with >=256KB transfers; a DMA completion semaphore fires
~1.5-2.5us after last byte under load; transpose-DMA runs ~26% slower
than straight DMA (not used).
"""

import numpy as np
import ml_dtypes
from contextlib import ExitStack

import concourse.bass as bass
import concourse.bacc as bacc
import concourse.tile as tile
from concourse import mybir
from concourse.bass_utils import run_bass_kernel_spmd

B, N, A = 4096, 16, 8
D_IN, H1, DP, DZ = 128, 64, 64, 64
NCORES = 8
BC = B // NCORES          # 512 envs per core
RC = BC * N               # 8192 obs rows per core
G = 4                     # chunks (128 envs each) per core
# cst cols (bf16): id128 | w1q 64 | wq_aug 2 | b1 | ones | q 64 | p 64
CW = 128 + 64 + 2 + 1 + 1 + 64 + 64   # = 324

F32 = mybir.dt.float32
BF16 = mybir.dt.bfloat16
ALU = mybir.AluOpType
AFT = mybir.ActivationFunctionType


def _build():
    nc = bacc.Bacc("TRN2", target_bir_lowering=False, debug=False)

    obs = nc.dram_tensor("obs", [RC, D_IN], BF16, kind="ExternalInput")
    cst = nc.dram_tensor("cst", [128, CW], BF16, kind="ExternalInput")
    xo = nc.dram_tensor("xo", [RC, N], BF16, kind="ExternalOutput")
    wo = nc.dram_tensor("wo", [RC, N], BF16, kind="ExternalOutput")

    with ExitStack() as ctx:
        tc = ctx.enter_context(tile.TileContext(nc))
        consts = ctx.enter_context(tc.tile_pool(name="consts", bufs=1))
        obsp = ctx.enter_context(tc.tile_pool(name="obsp", bufs=1))
        trp = ctx.enter_context(tc.tile_pool(name="trp", bufs=2))
        chp = ctx.enter_context(tc.tile_pool(name="chp", bufs=2))
        smal = ctx.enter_context(tc.tile_pool(name="smal", bufs=2))
        outp = ctx.enter_context(tc.tile_pool(name="outp", bufs=1))
        pmtp = ctx.enter_context(tc.tile_pool(name="pmtp", bufs=2, space="PSUM"))
        php = ctx.enter_context(tc.tile_pool(name="php", bufs=2, space="PSUM"))
        pwtp = ctx.enter_context(tc.tile_pool(name="pwtp", bufs=2, space="PSUM"))

        # ---- input DMAs ------------------------------------------------
        # ring A (sync):   cst, ch0, ch2a, ch3a, wo0, xo0, wo1
        # ring B (scalar): ch1, ch2b, ch3b, xo1
        cb = consts.tile([128, CW], BF16)
        nc.sync.dma_start(out=cb, in_=cst.ap())

        ov = obs.ap().rearrange("(g p n) f -> g p (n f)", g=G, p=128, n=N)
        ch = [obsp.tile([128, N, D_IN], BF16, name=f"ch{k}") for k in range(G)]
        flat = lambda t: t.rearrange("p n f -> p (n f)")
        half = lambda k, h: ch[k][:, 8 * h:8 * h + 8, :].rearrange(
            "p n f -> p (n f)")
        nc.sync.dma_start(out=flat(ch[0]), in_=ov[0])
        nc.scalar.dma_start(out=flat(ch[1]), in_=ov[1])
        i_ch2a = nc.sync.dma_start(out=half(2, 0), in_=ov[2][:, 0:1024])
        nc.scalar.dma_start(out=half(2, 1), in_=ov[2][:, 1024:2048])
        i_ch3a = nc.sync.dma_start(out=half(3, 0), in_=ov[3][:, 0:1024])
        i_ch3b = nc.scalar.dma_start(out=half(3, 1), in_=ov[3][:, 1024:2048])

        id128 = cb[:, 0:128]
        w1q = cb[:, 128:192]          # W1/16
        wq = cb[0:65, 192:194]        # [W2@wa | W2@wv64 ; b0 | b1v]
        b1c = cb[:, 194:195]          # b1 (rows 0:64)
        onec = cb[:, 195:196]         # 1.0 everywhere
        q_v = cb[:, 196:260].rearrange("p (g n) -> p g n", g=G)
        p_v = cb[:, 260:324].rearrange("p (g n) -> p g n", g=G)

        # warm the sigmoid table early (forces ACT_TABLE_LOAD up front)
        warm = consts.tile([1, 1], F32)
        nc.scalar.activation(out=warm, in_=cb[0:1, 195:196], func=AFT.Sigmoid)

        # h tiles carry a ones-row (row 64) so wq_aug applies the biases
        h_tiles = [chp.tile([65, 128], BF16, name="h_sb") for _ in range(2)]
        for t in h_tiles:
            nc.vector.tensor_copy(t[64:65, :],
                                  onec[64:65, :].broadcast_to([1, 128]))

        # ---- dot block (tiny, host-folded row sums); reduces must run
        # on DVE (GpSimd only reduces over partitions), A on GpSimd ----
        QS4 = smal.tile([128, G], F32, name="QS4")
        nc.vector.reduce_sum(out=QS4, in_=q_v, axis=mybir.AxisListType.X)
        PS4 = smal.tile([128, G], F32, name="PS4")
        nc.vector.reduce_sum(out=PS4, in_=p_v, axis=mybir.AxisListType.X)
        Abuf = smal.tile([128, G, N], F32, name="Abuf")
        nc.vector.scalar_tensor_tensor(
            out=Abuf, in0=q_v, scalar=-1.0,
            in1=QS4.unsqueeze(2).broadcast_to([128, G, N]),
            op0=ALU.mult, op1=ALU.add)

        # ---- per-chunk chain -------------------------------------------
        def tree(k):
            """DVE partial tree + accumulating PE transposes -> meanT."""
            t = ch[k]
            s1 = trp.tile([128, 8, D_IN], BF16, name="s1")
            nc.vector.tensor_add(s1, t[:, 0:8, :], t[:, 8:16, :])
            s2 = trp.tile([128, 4, D_IN], BF16, name="s2")
            nc.vector.tensor_add(s2, s1[:, 0:4, :], s1[:, 4:8, :])
            # remaining node-sum as accumulating PE matmuls against the
            # identity: pmt += s2_i^T @ I (real fp32 PSUM accumulation;
            # the is_transpose fast path does NOT accumulate)
            pmt = pmtp.tile([128, 128], F32, name="pmt")
            if k == 3:
                s3 = trp.tile([128, 2, D_IN], BF16, name="s3")
                nc.vector.tensor_add(s3, s2[:, 0:2, :], s2[:, 2:4, :])
                nc.tensor.matmul(pmt, lhsT=s3[:, 0, :], rhs=id128,
                                 start=True, stop=False)
                nc.tensor.matmul(pmt, lhsT=s3[:, 1, :], rhs=id128,
                                 start=False, stop=True)
            else:
                for i in range(4):
                    nc.tensor.matmul(pmt, lhsT=s2[:, i, :], rhs=id128,
                                     start=(i == 0), stop=(i == 3))
            meanT = chp.tile([128, 128], BF16, name="meanT")
            nc.scalar.activation(out=meanT, in_=pmt, func=AFT.Copy)
            return meanT

        def head(k, meanT, pwt, g2):
            ph = php.tile([64, 128], F32, name="ph")
            nc.tensor.matmul(ph, lhsT=w1q, rhs=meanT, start=True, stop=True)
            h_sb = h_tiles[k % 2]
            nc.scalar.activation(out=h_sb[0:64, :], in_=ph, func=AFT.Relu,
                                 bias=b1c[0:64])
            nc.tensor.matmul(pwt[:, g2, :], lhsT=h_sb, rhs=wq,
                             start=True, stop=True)

        def supergroup(j, pwt, veng):
            """post-matmul combine; smalls on `veng` (DVE or GpSimd
            via DVE for PSUM-reading steps)."""
            # leaky-relu: t1 = 0.01*a (PSUM read), wl = max(t1, a)
            t1 = smal.tile([128, 2, 1], F32, name="t1")
            nc.vector.tensor_scalar_mul(t1, pwt[:, :, 0:1], 0.01)
            wl = smal.tile([128, 2, 1], F32, name="wl")
            nc.vector.tensor_max(wl, t1, pwt[:, :, 0:1])
            nc.scalar.activation(out=wl, in_=wl, func=AFT.Sigmoid)
            Bt = smal.tile([128, 2, 1], F32, name="Bt")
            nc.vector.tensor_add(Bt, pwt[:, :, 1:2],
                                 PS4[:, 2 * j:2 * j + 2].unsqueeze(2))
            m = smal.tile([128, 2, N], F32, name="m")
            veng.tensor_mul(m, Abuf[:, 2 * j:2 * j + 2, :],
                            wl.broadcast_to([128, 2, N]))
            xv = smal.tile([128, 2, N], F32, name="xv")
            veng.tensor_add(xv, m, Bt.broadcast_to([128, 2, N]))
            xbig = outp.tile([128, 2 * N * N], BF16, name=f"xbig{j}")
            nc.vector.tensor_copy(
                xbig.rearrange("p (g d j) -> p g d j", g=2, d=N),
                xv.unsqueeze(2).broadcast_to([128, 2, N, N]))
            wbig = outp.tile([128, 2 * N * N], BF16, name=f"wbig{j}")
            nc.scalar.activation(
                out=wbig.rearrange("p (g dj) -> p g dj", g=2),
                in_=wl.broadcast_to([128, 2, N * N]), func=AFT.Copy)
            return wbig, xbig

        wo_v = wo.ap().rearrange("(jj g2 p d) j -> jj p g2 (d j)",
                                 jj=2, g2=2, p=128, d=N)
        xo_v = xo.ap().rearrange("(jj g2 p d) j -> jj p g2 (d j)",
                                 jj=2, g2=2, p=128, d=N)
        g2v = lambda t: t.rearrange("p (g2 dj) -> p g2 dj", g2=2)

        pwt0 = pwtp.tile([128, 2, 2], F32, name="pwt0")
        pwt1 = pwtp.tile([128, 2, 2], F32, name="pwt1")

        head(0, tree(0), pwt0, 0)
        head(1, tree(1), pwt0, 1)
        wbig0, xbig0 = supergroup(0, pwt0, nc.gpsimd)
        i_wo0 = nc.sync.dma_start(out=wo_v[0], in_=g2v(wbig0))
        i_xo0 = nc.sync.dma_start(out=xo_v[0], in_=g2v(xbig0))

        head(2, tree(2), pwt1, 0)
        head(3, tree(3), pwt1, 1)
        wbig1, xbig1 = supergroup(1, pwt1, nc.vector)
        i_wo1 = nc.sync.dma_start(out=wo_v[1], in_=g2v(wbig1))
        i_xo1 = nc.scalar.dma_start(out=xo_v[1], in_=g2v(xbig1))

        # keep output triggers behind the last input trigger on each ring
        prev = i_ch3a
        for di in (i_wo0, i_xo0, i_wo1):
            tile.add_dep_helper(di.ins, prev.ins, sync=False,
                                reason="sync outputs after inputs, in order")
            prev = di
        tile.add_dep_helper(i_xo1.ins, i_ch3b.ins, sync=False,
                            reason="scalar output after inputs")

    nc.compile()
    return nc


_NC_CACHE = {}


def _get_nc():
    if "nc" not in _NC_CACHE:
        _NC_CACHE["nc"] = _build()
    return _NC_CACHE["nc"]


def _make_in_maps(inputs):
    bf = ml_dtypes.bfloat16
    obs = np.ascontiguousarray(np.asarray(inputs["obs"], np.float32)).astype(bf)
    pol0 = np.asarray(inputs["policies"], np.float32)
    act0 = np.asarray(inputs["actions"], np.float32)
    W1 = np.asarray(inputs["W1"], np.float32)
    b1 = np.asarray(inputs["b1"], np.float32)
    W2 = np.asarray(inputs["W2"], np.float32)
    b2 = np.asarray(inputs["b2"], np.float32)
    Wfc = np.asarray(inputs["Wfc"], np.float32)
    Wattn = np.asarray(inputs["Wattn"], np.float32)
    Wv = np.asarray(inputs["Wv"], np.float32)
    bv = np.asarray(inputs["bv"], np.float32)

    wa = (Wfc @ (Wattn[:DZ] + Wattn[DZ:]))[:, 0]     # [64]
    wvy = Wv[DP:, 0]                                  # [8]
    wv64 = Wv[:DP, 0]

    # host-folded per-(env,node) dot scalars, pre-divided by N
    qv = ((act0 - pol0) * wvy).sum(-1) / float(N)     # [B*N] f32
    pv = (pol0 * wvy).sum(-1) / float(N)

    base = np.zeros((128, 196), np.float32)
    base[:, 0:128] = np.eye(128, dtype=np.float32)
    base[:, 128:192] = W1 / float(N)
    base[0:64, 192] = W2 @ wa
    base[0:64, 193] = W2 @ wv64
    base[64, 192] = float(b2 @ wa)            # b0 via ones-row
    base[64, 193] = float(b2 @ wv64 + bv[0])  # b1v via ones-row
    base[0:64, 194] = b1
    base[:, 195] = 1.0

    in_maps = []
    for c in range(NCORES):
        # q/p of env 128g+p, node n (within core c) at cols [196+, 260+)
        qc = qv[c * RC:(c + 1) * RC].reshape(G, 128, N).transpose(1, 0, 2)
        pc = pv[c * RC:(c + 1) * RC].reshape(G, 128, N).transpose(1, 0, 2)
        cst_c = np.concatenate(
            [base, qc.reshape(128, 64), pc.reshape(128, 64)],
            axis=1).astype(bf)
        in_maps.append({
            "obs": obs[c * RC:(c + 1) * RC],
            "cst": np.ascontiguousarray(cst_c),
        })
    return in_maps


# Test-harness knobs (the grader just calls kernel() with defaults).
TRACE = False
TRACE_KWARGS = {}
LAST_RESULT = None


def kernel(**inputs):
    global LAST_RESULT
    nc = _get_nc()
    in_maps = _make_in_maps(inputs)
    res = run_bass_kernel_spmd(nc, in_maps, core_ids=list(range(NCORES)),
                               trace=TRACE, **TRACE_KWARGS)
    LAST_RESULT = res
    x = np.concatenate([np.asarray(r["xo"], np.float32)
                        for r in res.results], axis=0).reshape(B * N, N, 1)
    w = np.concatenate([np.asarray(r["wo"], np.float32)
                        for r in res.results], axis=0).reshape(B * N, N, 1)
    return x, w
